# revision 21
# baseline (speedup 1.0000x reference)
"""Distance-scorer Bass kernel for 8 Trainium2 NeuronCores.

Math: score[b,k] = W2 . relu(W1[bin,:] + x*W1[50] + y*W1[51]
                             + ego[b] @ W1[52:56] + b1) + b2
with s = x^2 + y^2, bin = clip(floor(fp32(sqrt(s)/50*50)), 0, 49).

Per core (data-parallel B-shard of 256 rows):
  - s = x^2+y^2 on DVE (fp32, bit-identical to the reference).
  - A per-row selector matmul (fp32, exact) replicates the row's s across
    114 PSUM partitions (50 thresholds x 2 element-halves + padding).
  - DVE tensor_scalar(is_ge, per-partition threshold) builds the cumulative
    "staircase" of the histogram one-hot; thresholds are the exact fp32 cut
    points of the reference binning, so binning matches bit-for-bit.
  - One f32r matmul against the Abel-differenced table computes
    T[bin] + x*wx + y*wy (64 hidden dims, 2 elements packed per column).
  - ScalarE applies the per-row ego/b1 bias + ReLU; |W2| is folded into all
    weights so a +/-1 f32r reduction matmul yields the scores.
"""

import numpy as np

N_CORES = 8
B, K = 2048, 1024
NB = 50
D = 64
BS = B // N_CORES  # 256 rows/core
F = K // 2  # 512 moving columns; 2 elements packed per column
RPB = 64  # rows per s-block
GRP = 16  # rows per stair tensor
SELW = 114  # selector output rows: 50 A + 14 pad + 50 B
KC = 118  # mm1 contraction: 114 stair rows + 4 xy rows


def _exact_thresholds():
    """t[j] = smallest fp32 s with floor(fp32(fp32(sqrt(s))/50)*50) >= j."""

    def bin_of(s_u32):
        s = np.uint32(s_u32).view(np.float32)
        d = np.sqrt(s, dtype=np.float32)
        v = np.float32(np.float32(d / np.float32(50.0)) * np.float32(50.0))
        return int(np.floor(v))

    ts = np.empty(NB, dtype=np.float32)
    ts[0] = -3.0e38
    for j in range(1, NB):
        lo = np.uint32(0)
        hi = np.float32(2.6e9).view(np.uint32)
        assert bin_of(hi) >= j
        while int(hi) - int(lo) > 1:
            mid = np.uint32((int(lo) + int(hi)) // 2)
            if bin_of(mid) >= j:
                hi = mid
            else:
                lo = mid
        ts[j] = np.uint32(hi).view(np.float32)
    return ts


def _consts(W1, b1, W2, b2):
    W1 = np.asarray(W1, np.float32)
    b1 = np.asarray(b1, np.float32)
    W2 = np.asarray(W2, np.float32)
    absw2 = np.abs(W2[:, 0])
    sgn = np.where(W2[:, 0] < 0, -1.0, 1.0).astype(np.float32)

    Tt = W1[:NB] * absw2[None, :]
    dT = Tt.copy()
    dT[1:] -= Tt[:-1]
    wx = W1[50] * absw2
    wy = W1[51] * absw2

    smm1 = np.zeros((KC, 128), np.float32)
    smm1[0:50, 0:64] = dT
    smm1[64:114, 64:128] = dT
    smm1[114, 0:64] = wx
    smm1[115, 0:64] = wy
    smm1[116, 64:128] = wx
    smm1[117, 64:128] = wy

    import ml_dtypes
    smm2 = np.zeros((128, 2), ml_dtypes.bfloat16)
    smm2[0:64, 0] = sgn.astype(ml_dtypes.bfloat16)
    smm2[64:128, 1] = sgn.astype(ml_dtypes.bfloat16)

    wego = np.zeros((5, 128), np.float32)
    for i in range(4):
        wego[i, 0:64] = W1[52 + i] * absw2
        wego[i, 64:128] = W1[52 + i] * absw2
    wego[4, 0:64] = b1 * absw2
    wego[4, 64:128] = b1 * absw2

    t = _exact_thresholds()
    tcol = np.full((SELW, 1), 3.0e38, np.float32)
    tcol[0:50, 0] = t
    tcol[64:114, 0] = t

    return dict(smm1=smm1, smm2=smm2, wego=wego, tcol=tcol,
                b2=float(np.asarray(b2, np.float32).reshape(-1)[0]))


def _build():
    import concourse.bass as bass  # noqa: F401
    import concourse.mybir as mybir
    from concourse import bacc
    from concourse.tile import TileContext

    f32 = mybir.dt.float32
    f32r = mybir.dt.float32r
    bf16 = mybir.dt.bfloat16
    Relu = mybir.ActivationFunctionType.Relu

    nc = bacc.Bacc("TRN2", target_bir_lowering=False, debug=False,
                   num_devices=N_CORES)

    gpx_d = nc.declare_dram_parameter("gpx", [BS, K], f32, isOutput=False)
    gpy_d = nc.declare_dram_parameter("gpy", [BS, K], f32, isOutput=False)
    ego_d = nc.declare_dram_parameter("ego", [BS, 4], f32, isOutput=False)
    smm1_d = nc.declare_dram_parameter("smm1", [KC, 128], f32r, isOutput=False)
    smm2_d = nc.declare_dram_parameter("smm2", [128, 2], bf16, isOutput=False)
    gpxr_d = nc.declare_dram_parameter("gpxr", [BS, K], f32r, isOutput=False)
    gpyr_d = nc.declare_dram_parameter("gpyr", [BS, K], f32r, isOutput=False)
    wego_d = nc.declare_dram_parameter("wego", [5, 128], f32, isOutput=False)
    tcol_d = nc.declare_dram_parameter("tcol", [SELW, 1], f32, isOutput=False)
    b2_d = nc.declare_dram_parameter("b2c", [128, 1], f32, isOutput=False)
    sc_d = nc.declare_dram_parameter("scores", [BS, K], f32, isOutput=True)

    with TileContext(nc) as tc:
        with (
            tc.tile_pool(name="consts", bufs=1) as cpool,
            tc.tile_pool(name="work", bufs=3) as wpool,
            tc.tile_pool(name="stairp", bufs=6) as spool,
            tc.tile_pool(name="hrelu", bufs=6) as hpool,
            tc.tile_pool(name="mps", bufs=6) as mpspool,
            tc.tile_pool(name="sdram", bufs=2, space="DRAM") as dpool,
            tc.tile_pool(name="p1", bufs=4, space="PSUM") as p1pool,
            tc.tile_pool(name="p2", bufs=2, space="PSUM") as p2pool,
        ):
            smm1_s = cpool.tile([KC, 128], f32r, tag="smm1")
            smm2_s = cpool.tile([128, 2], bf16, tag="smm2")
            wego_s = cpool.tile([5, 128], f32, tag="wego")
            tcol_s = cpool.tile([SELW, 1], f32, tag="tcol")
            b2_s = cpool.tile([128, 1], f32, tag="b2c")
            nc.sync.dma_start(out=smm1_s[:], in_=smm1_d[:])
            nc.sync.dma_start(out=smm2_s[:], in_=smm2_d[:])
            nc.sync.dma_start(out=wego_s[:], in_=wego_d[:])
            nc.sync.dma_start(out=tcol_s[:], in_=tcol_d[:])
            nc.sync.dma_start(out=b2_s[:], in_=b2_d[:])

            # per-row ego bias c2[:, r] = [ego[r]@W1e + b1; same] * |W2|
            ego5 = cpool.tile([5, BS], f32, tag="ego5")
            nc.vector.memset(ego5[:], 1.0)
            nc.sync.dma_start(out=ego5[0:4, :], in_=ego_d[:].rearrange("b i -> i b"))
            c2_psum = p2pool.tile([128, BS], f32, tag="p2")
            nc.tensor.matmul(c2_psum[:], lhsT=wego_s[:], rhs=ego5[:],
                             start=True, stop=True)
            c2_s = cpool.tile([128, BS], f32, tag="c2")
            nc.scalar.copy(c2_s[:], c2_psum[:])

            p2 = None
            scr = None
            for blk in range(BS // RPB):
                r0 = blk * RPB
                gpxp = wpool.tile([128, F], f32, tag="gpxp")
                gpyp = wpool.tile([128, F], f32, tag="gpyp")
                nc.sync.dma_start(
                    out=gpxp[:],
                    in_=gpx_d[r0 : r0 + RPB].rearrange("b (h k) -> (b h) k", h=2))
                nc.sync.dma_start(
                    out=gpyp[:],
                    in_=gpy_d[r0 : r0 + RPB].rearrange("b (h k) -> (b h) k", h=2))
                xx = wpool.tile([128, F], f32, tag="xx")
                s_pair = wpool.tile([128, F], f32, tag="s_pair")
                nc.vector.tensor_tensor(out=xx[:], in0=gpxp[:], in1=gpxp[:],
                                        op=mybir.AluOpType.mult)
                nc.vector.tensor_tensor(out=s_pair[:], in0=gpyp[:], in1=gpyp[:],
                                        op=mybir.AluOpType.mult)
                nc.vector.tensor_tensor(out=s_pair[:], in0=s_pair[:], in1=xx[:],
                                        op=mybir.AluOpType.add)
                s_dram = dpool.tile([RPB, K], f32, tag="sdram")
                nc.sync.dma_start(
                    out=s_dram[:].rearrange("b (h k) -> (b h) k", h=2),
                    in_=s_pair[:])

                for p in range(RPB // 2):
                        gA = 2 * p                  # row index within s-block
                        gB = gA + 1
                        rA = r0 + gA                # global rows
                        rB = rA + 1
                        stair = spool.tile([KC, 2 * F], f32r, tag="stairB")
                        # xy rows: 114/116 = x (A/B half), 115/117 = y
                        for c, gsrc in ((0, gpxr_d), (1, gpyr_d)):
                            src = gsrc[rA : rA + 2]
                            src_b = bass.AP(
                                tensor=src.tensor, offset=src.offset,
                                ap=[[F, 2], [K, 2], [1, F]])
                            dst = stair[114 + c : 118 : 2, :]
                            dst_b = bass.AP(
                                tensor=dst.tensor, offset=dst.offset,
                                ap=[list(dst.ap[0]), [F, 2], [1, F]])
                            eng = nc.scalar if c == 0 else nc.gpsimd
                            eng.dma_start(out=dst_b, in_=src_b)
                        mp = mpspool.tile([SELW, 2 * F], f32, tag="mp")
                        # broadcast s of rows (gA, gB) from DRAM scratch:
                        # A-halves to partitions 0:50, B-halves to 64:114
                        sd = s_dram[gA : gA + 2]
                        for base, off in ((0, 0), (64, F)):
                            src_ap = bass.AP(
                                tensor=sd.tensor, offset=sd.offset + off,
                                ap=[[0, 50], [K, 2], [1, F]])
                            eng = nc.sync if base == 0 else nc.gpsimd
                            eng.dma_start(out=mp[base : base + 50, :],
                                          in_=src_ap)
                        nc.vector.tensor_scalar(
                            out=stair[0:SELW, :], in0=mp[:],
                            scalar1=tcol_s[:], scalar2=None,
                            op0=mybir.AluOpType.is_ge)
                        p1a = p1pool.tile([128, F], f32, tag="p1")
                        nc.tensor.matmul(
                            p1a[:], lhsT=smm1_s[:],
                            rhs=stair[0:KC, 0:F],
                            start=True, stop=True)
                        p1b = p1pool.tile([128, F], f32, tag="p1")
                        nc.tensor.matmul(
                            p1b[:], lhsT=smm1_s[:],
                            rhs=stair[0:KC, F : 2 * F],
                            start=True, stop=True)
                        for p1x, r in ((p1a, rA), (p1b, rB)):
                            hr = hpool.tile([128, F], bf16, tag="hr")
                            nc.scalar.activation(
                                hr[:], p1x[:], Relu,
                                bias=c2_s[:, r : r + 1], scale=1.0)
                            a = r % 3
                            if a == 0:
                                p2 = p2pool.tile([128, F], f32, tag="p2")
                            nc.tensor.matmul(
                                p2[32 * a : 32 * a + 2, :],
                                lhsT=smm2_s[:],
                                rhs=hr[:], start=True, stop=True)
                            if a == 2 or r == BS - 1:
                                scr = wpool.tile([128, F], f32, tag="scr")
                                nc.vector.tensor_scalar_add(scr[:], p2[:],
                                                            b2_s[:, 0:1])
                                for aa in range(a + 1):
                                    nc.gpsimd.dma_start(
                                        out=sc_d[r - a + aa].rearrange(
                                            "(h k) -> h k", h=2),
                                        in_=scr[32 * aa : 32 * aa + 2, :])

    nc.finalize()
    return nc


_CACHE = {}


def make_in_maps(goal_positions, ego_state, W1, b1, W2, b2):
    gp = np.asarray(goal_positions, np.float32)
    gpx = np.ascontiguousarray(gp[..., 0])
    gpy = np.ascontiguousarray(gp[..., 1])
    ego = np.ascontiguousarray(np.asarray(ego_state, np.float32))
    c = _consts(W1, b1, W2, b2)
    b2col = np.full((128, 1), c["b2"], np.float32)
    in_maps = []
    for i in range(N_CORES):
        in_maps.append({
            "gpx": gpx[i * BS : (i + 1) * BS],
            "gpy": gpy[i * BS : (i + 1) * BS],
            "gpxr": gpx[i * BS : (i + 1) * BS],
            "gpyr": gpy[i * BS : (i + 1) * BS],
            "ego": ego[i * BS : (i + 1) * BS],
            "smm1": c["smm1"], "smm2": c["smm2"], "wego": c["wego"],
            "tcol": c["tcol"], "b2c": b2col,
        })
    return in_maps


def kernel(goal_positions, ego_state, W1, b1, W2, b2):
    from concourse.bass_utils import run_bass_kernel_spmd

    if "nc" not in _CACHE:
        _CACHE["nc"] = _build()
    nc = _CACHE["nc"]

    in_maps = make_in_maps(goal_positions, ego_state, W1, b1, W2, b2)
    res = run_bass_kernel_spmd(nc, in_maps, core_ids=list(range(N_CORES)))
    out = np.concatenate([res.results[i]["scores"] for i in range(N_CORES)],
                         axis=0)
    return out.astype(np.float32)
